# revision 9
# baseline (speedup 1.0000x reference)
"""Trainium2 Bass kernel for BoostedPointPairNet2.

Model (per (b, d) group, m = 128 points, din = 3):
  H1(i,j) = relu(W1A @ x_j + W1B @ x_i + b1)          (64)
  H2(i,j) = relu(W2 @ H1 + b2)                        (128)
  G(i,j)  = W3 @ H2                                    (256, b3 deferred)
  P       = max_{i,j} G + b3                           (256)
  Y       = V3 @ relu(V2 @ relu(V1 @ P + c1) + c2) + c3  (40)
  out[b]  = max_d Y[b, d]

Sharding: 16 (b, d) groups over 8 cores, 2 groups per core. Weights
replicated. Each core returns its two groups' Y rows; the host does the
final max over d (the trivial "all-gather" of a (b, 40) output).

Per-core dataflow ("stacked pairs" layout): channels of two j-values are
stacked on the 128 SBUF partitions (j even -> partitions 0-63, j odd ->
64-127), so layer-1 (only 64 channels) runs at full partition width as ONE
fused DVE tensor_scalar op per j-pair: relu(v_stacked + (u_j + b1)).
L2 unstacks via block-padded weights [W2^T;0] / [0;W2^T]; L3 streams H2
through W3^T halves into PSUM.

The G max-reduction is the bottleneck: PSUM can only be read by the DVE
(0.96 GHz, the only engine with max) and ACT (1.2 GHz, no max). So G
tiles alternate between (a) direct DVE reduce_max from PSUM and (b) ACT
copy/cast to bf16 SBUF + DVE running tensor_tensor max at the 2x bf16
mode - balancing the two engines. Matmuls and big activations run in
bf16 (fp32 PSUM accumulation); the tiny F-MLP runs in fp32.
"""

import numpy as np
import ml_dtypes

import bass_rust
import concourse.bass as bass
import concourse.mybir as mybir
from concourse.tile import TileContext
from concourse.bass_utils import run_bass_kernel_spmd

BF16 = ml_dtypes.bfloat16
F32 = np.float32
DT = mybir.dt
ALU = mybir.AluOpType
AX = mybir.AxisListType
RELU = mybir.ActivationFunctionType.Relu

N_CORES = 8
B, N, DIN = 4, 512, 3
D = 4                    # boost factor
M = N // D               # 128 points per group
GROUPS_PER_CORE = 2
JP = M // 2              # 64 stacked j-pairs per group
NPAIR = JP // 4          # 16 pipeline iterations per group (4 jp each)
NGT = 2 * NPAIR          # 32 G-psum tiles per group
# G tiles with (t % DIRECT_MOD == 0) are reduced directly from PSUM by the
# DVE; the rest are ACT-copied to bf16 SBUF and running-maxed by the DVE.
DIRECT_MOD = 3
N_DIRECT = (NGT + DIRECT_MOD - 1) // DIRECT_MOD


# ---------------------------------------------------------------------------
# Workaround: this walrus build accepts at most ONE sync wait per instruction
# ("Too many sync wait commands"), while Tile emits up to 3. Hoist extra
# waits onto same-engine nop instructions inserted just before the offender
# (engines execute their queue in order, so the AND-wait semantics hold).
# ---------------------------------------------------------------------------
def _split_multi_waits(nc):
    seq = 0
    for fn in nc.m.functions:
        for bb in fn.blocks:
            new = []
            changed = False
            for ins in bb.instructions:
                si = ins.sync_info
                waits = list(si.on_wait) if si is not None and si.on_wait else []
                if len(waits) > 1:
                    changed = True
                    for w in waits[:-1]:
                        seq += 1
                        new.append(
                            mybir.InstNoOp(
                                name=f"I-wsplit-{seq}",
                                engine=ins.engine,
                                sync_info=bass_rust.SyncInfo(
                                    on_wait=[w], on_update=[]
                                ),
                            )
                        )
                    ins.sync_info = bass_rust.SyncInfo(
                        on_wait=[waits[-1]], on_update=list(si.on_update or [])
                    )
                new.append(ins)
            if changed:
                bb.instructions = new


# ---------------------------------------------------------------------------
# Device program
# ---------------------------------------------------------------------------
def _build_program():
    nc = bass.Bass(
        "TRN2", target_bir_lowering=False, debug=False, num_devices=N_CORES
    )

    xt = nc.declare_dram_parameter(
        "xt", [GROUPS_PER_CORE, DIN, M], DT.bfloat16, isOutput=False
    )
    # col 0:128 w2a=[W2T;0], 128:256 w2b=[0;W2T], 256:384 w3a, 384:512 w3b
    wblob = nc.declare_dram_parameter("wblob", [128, 512], DT.bfloat16, isOutput=False)
    # col 0:128 w1a_even=[W1AT|0], 128:256 w1a_odd=[0|W1AT], 256:384 w1b2=[W1BT|W1BT]
    w1blob = nc.declare_dram_parameter("w1blob", [DIN, 384], DT.bfloat16, isOutput=False)
    # v1t (2x512) | v2t (4x256) | v3t (2x40)
    vblob = nc.declare_dram_parameter("vblob", [128, 2128], DT.bfloat16, isOutput=False)
    # col 0 b1st, 1 b2c, 2:4 b3_2, 4:8 c1_4, 8:10 c2_2, 10 c3 (rows 0:40)
    cblob = nc.declare_dram_parameter("cblob", [128, 11], DT.float32, isOutput=False)
    y_out = nc.declare_dram_parameter(
        "y", [GROUPS_PER_CORE, 40], DT.float32, isOutput=True
    )

    with TileContext(nc) as tc:
        with (
            tc.tile_pool(name="singles", bufs=1) as singles,
            tc.tile_pool(name="xtp", bufs=2) as xtp,
            tc.tile_pool(name="v2p", bufs=2) as v2pool,
            tc.tile_pool(name="uup", bufs=2) as uupool,
            tc.tile_pool(name="h1p", bufs=3) as h1pool,
            tc.tile_pool(name="h2p", bufs=3) as h2pool,
            tc.tile_pool(name="gcp", bufs=4) as gcpool,
            tc.tile_pool(name="raccp", bufs=2) as raccpool,
            tc.tile_pool(name="fmlp", bufs=8) as fmlp,
            tc.tile_pool(name="psum", bufs=2, space="PSUM") as psum,
        ):
            # ---- load inputs; issue order = need order (2 HWDGE queues) ----
            sb_xts = []
            for g in range(GROUPS_PER_CORE):
                t = xtp.tile([DIN, M], DT.bfloat16)
                sb_xts.append(t)
            nc.sync.dma_start(out=sb_xts[0], in_=xt[0])
            sb_w1 = singles.tile([DIN, 384], DT.bfloat16, tag="w1blob")
            nc.sync.dma_start(out=sb_w1, in_=w1blob[:, :])
            sb_c = singles.tile([128, 11], DT.float32, tag="cblob")
            nc.scalar.dma_start(out=sb_c, in_=cblob[:, :])
            sb_w = singles.tile([128, 512], DT.bfloat16, tag="wblob")
            nc.sync.dma_start(out=sb_w, in_=wblob[:, :])
            sb_v = singles.tile([128, 2128], DT.bfloat16, tag="vblob")
            nc.scalar.dma_start(out=sb_v, in_=vblob[:, :])
            nc.sync.dma_start(out=sb_xts[1], in_=xt[1])

            sb_w2a, sb_w2b = sb_w[:, 0:128], sb_w[:, 128:256]
            sb_w3a, sb_w3b = sb_w[:, 256:384], sb_w[:, 384:512]
            sb_w1a_e, sb_w1a_o = sb_w1[:, 0:128], sb_w1[:, 128:256]
            sb_w1b2 = sb_w1[:, 256:384]
            sb_b1st = sb_c[:, 0:1]
            sb_b2c = sb_c[:, 1:2]
            sb_b3_2 = sb_c[:, 2:4]
            sb_c1_4 = sb_c[:, 4:8]
            sb_c2_2 = sb_c[:, 8:10]
            sb_c3c = sb_c[0:40, 10:11]

            def v1t(k):  # [128, 512] f32, k in 0..1
                return sb_v[:, 512 * k : 512 * (k + 1)]

            def v2t(k):  # [128, 256] f32, k in 0..3
                return sb_v[:, 1024 + 256 * k : 1024 + 256 * (k + 1)]

            def v3t(k):  # [128, 40] f32, k in 0..1
                return sb_v[:, 2048 + 40 * k : 2048 + 40 * (k + 1)]

            for g in range(GROUPS_PER_CORE):
                # ---- per-group prep: stacked v (V2) and u+b1 (UU) ----
                sb_xt = sb_xts[g]
                xt_eo = sb_xt.rearrange("k (j two) -> k two j", two=2)

                v2ps = psum.tile([128, M], DT.float32, tag="l2")
                nc.tensor.matmul(v2ps, lhsT=sb_w1b2, rhs=sb_xt, start=True, stop=True)
                uups = psum.tile([128, JP], DT.float32, tag="g")
                nc.tensor.matmul(
                    uups, lhsT=sb_w1a_e, rhs=xt_eo[:, 0, :], start=True, stop=False
                )
                nc.tensor.matmul(
                    uups, lhsT=sb_w1a_o, rhs=xt_eo[:, 1, :], start=False, stop=True
                )
                sb_v2 = v2pool.tile([128, M], DT.bfloat16)
                nc.vector.tensor_copy(out=sb_v2, in_=v2ps)
                sb_uu = uupool.tile([128, JP], DT.float32)
                nc.vector.tensor_scalar_add(out=sb_uu, in0=uups, scalar1=sb_b1st)

                racc = raccpool.tile([128, 2, N_DIRECT], DT.float32)
                rbs = []
                for h in range(2):
                    rb = raccpool.tile([128, 1024], DT.bfloat16, tag=f"rb{h}")
                    nc.gpsimd.memset(rb, -1e30)
                    rbs.append(rb)

                # ---- main pairwise pipeline: 4 j-pairs per iteration ----
                for it in range(NPAIR):
                    h1 = h1pool.tile([128, 512], DT.bfloat16)
                    for jj in range(4):
                        jp = it * 4 + jj
                        nc.vector.tensor_scalar(
                            out=h1[:, jj * M : (jj + 1) * M],
                            in0=sb_v2,
                            scalar1=sb_uu[:, jp : jp + 1],
                            scalar2=0.0,
                            op0=ALU.add,
                            op1=ALU.max,
                        )
                    # L2: weight-grouped matmuls into one 2-bank psum tile
                    l2ps = psum.tile([128, 1024], DT.float32, tag="l2")
                    nc.tensor.matmul(
                        l2ps[:, 0:256], lhsT=sb_w2a, rhs=h1[:, 0:256],
                        start=True, stop=True,
                    )
                    nc.tensor.matmul(
                        l2ps[:, 512:768], lhsT=sb_w2a, rhs=h1[:, 256:512],
                        start=True, stop=True,
                    )
                    nc.tensor.matmul(
                        l2ps[:, 256:512], lhsT=sb_w2b, rhs=h1[:, 0:256],
                        start=True, stop=True,
                    )
                    nc.tensor.matmul(
                        l2ps[:, 768:1024], lhsT=sb_w2b, rhs=h1[:, 256:512],
                        start=True, stop=True,
                    )
                    h2 = h2pool.tile([128, 1024], DT.bfloat16)
                    nc.scalar.activation(
                        out=h2, in_=l2ps, func=RELU, bias=sb_b2c, scale=1.0
                    )
                    # L3: weight-grouped into two G tiles (1024 pairs total)
                    gpa = psum.tile([128, 2, 512], DT.float32, tag="g")
                    gpb = psum.tile([128, 2, 512], DT.float32, tag="g")
                    nc.tensor.matmul(
                        gpa[:, 0, :], lhsT=sb_w3a, rhs=h2[:, 0:512],
                        start=True, stop=True,
                    )
                    nc.tensor.matmul(
                        gpb[:, 0, :], lhsT=sb_w3a, rhs=h2[:, 512:1024],
                        start=True, stop=True,
                    )
                    nc.tensor.matmul(
                        gpa[:, 1, :], lhsT=sb_w3b, rhs=h2[:, 0:512],
                        start=True, stop=True,
                    )
                    nc.tensor.matmul(
                        gpb[:, 1, :], lhsT=sb_w3b, rhs=h2[:, 512:1024],
                        start=True, stop=True,
                    )
                    for half, gp in enumerate((gpa, gpb)):
                        t = it * 2 + half
                        if t % DIRECT_MOD == 0:
                            nc.vector.reduce_max(
                                out=racc[:, :, t // DIRECT_MOD : t // DIRECT_MOD + 1],
                                in_=gp, axis=AX.X,
                            )
                        else:
                            gc = gcpool.tile([128, 1024], DT.bfloat16)
                            nc.scalar.copy(
                                out=gc, in_=gp.rearrange("p a b -> p (a b)")
                            )
                            rb = rbs[t % 2]
                            nc.vector.tensor_tensor(
                                out=rb, in0=gc, in1=rb, op=ALU.max
                            )

                # ---- P = max over accumulators, + b3; F MLP (fp32, N=1) ----
                pm1 = fmlp.tile([128, 2], DT.float32, tag="pm1")
                nc.vector.reduce_max(out=pm1, in_=racc, axis=AX.X)
                nc.vector.tensor_tensor(
                    out=rbs[0], in0=rbs[0], in1=rbs[1], op=ALU.max
                )
                pm2 = fmlp.tile([128, 2], DT.float32, tag="pm2")
                nc.vector.reduce_max(
                    out=pm2, in_=rbs[0].rearrange("p (a b) -> p a b", a=2), axis=AX.X
                )
                pmx = fmlp.tile([128, 2], DT.float32, tag="pmx")
                nc.vector.tensor_tensor(out=pmx, in0=pm1, in1=pm2, op=ALU.max)
                pb = fmlp.tile([128, 2], DT.bfloat16, tag="pb")
                nc.vector.tensor_tensor(out=pb, in0=pmx, in1=sb_b3_2, op=ALU.add)

                y1ps = psum.tile([128, 4], DT.float32, tag="l2")
                for mm in range(4):
                    for kk in range(2):
                        nc.tensor.matmul(
                            y1ps[:, mm : mm + 1],
                            lhsT=v1t(kk)[:, mm * 128 : (mm + 1) * 128],
                            rhs=pb[:, kk : kk + 1],
                            start=(kk == 0),
                            stop=(kk == 1),
                        )
                y1pre = fmlp.tile([128, 4], DT.float32, tag="y1pre")
                nc.vector.tensor_tensor(out=y1pre, in0=y1ps, in1=sb_c1_4, op=ALU.add)
                y1 = fmlp.tile([128, 4], DT.bfloat16, tag="y1")
                nc.vector.tensor_scalar_max(out=y1, in0=y1pre, scalar1=0.0)

                y2ps = psum.tile([128, 2], DT.float32, tag="l2")
                for mm in range(2):
                    for kk in range(4):
                        nc.tensor.matmul(
                            y2ps[:, mm : mm + 1],
                            lhsT=v2t(kk)[:, mm * 128 : (mm + 1) * 128],
                            rhs=y1[:, kk : kk + 1],
                            start=(kk == 0),
                            stop=(kk == 3),
                        )
                y2pre = fmlp.tile([128, 2], DT.float32, tag="y2pre")
                nc.vector.tensor_tensor(out=y2pre, in0=y2ps, in1=sb_c2_2, op=ALU.add)
                y2 = fmlp.tile([128, 2], DT.bfloat16, tag="y2")
                nc.vector.tensor_scalar_max(out=y2, in0=y2pre, scalar1=0.0)

                y3ps = psum.tile([40, 1], DT.float32, tag="g")
                for kk in range(2):
                    nc.tensor.matmul(
                        y3ps,
                        lhsT=v3t(kk)[:, 0:40],
                        rhs=y2[:, kk : kk + 1],
                        start=(kk == 0),
                        stop=(kk == 1),
                    )
                y3 = fmlp.tile([40, 1], DT.float32, tag="y3")
                nc.vector.tensor_scalar_add(out=y3, in0=y3ps, scalar1=sb_c3c)
                nc.sync.dma_start(out=y_out[g, :], in_=y3)

    _split_multi_waits(nc)
    return nc


# ---------------------------------------------------------------------------
# Host side
# ---------------------------------------------------------------------------
_NC_CACHE = None


def _get_program():
    global _NC_CACHE
    if _NC_CACHE is None:
        _NC_CACHE = _build_program()
    return _NC_CACHE


def _make_in_maps(inputs):
    X = np.asarray(inputs["X"], F32)
    W1 = np.asarray(inputs["W1"], F32)
    b1 = np.asarray(inputs["b1"], F32)
    W2 = np.asarray(inputs["W2"], F32)
    b2 = np.asarray(inputs["b2"], F32)
    W3 = np.asarray(inputs["W3"], F32)
    b3 = np.asarray(inputs["b3"], F32)
    V1 = np.asarray(inputs["V1"], F32)
    c1 = np.asarray(inputs["c1"], F32)
    V2 = np.asarray(inputs["V2"], F32)
    c2 = np.asarray(inputs["c2"], F32)
    V3 = np.asarray(inputs["V3"], F32)
    c3 = np.asarray(inputs["c3"], F32)

    W1A, W1B = W1[:, :DIN], W1[:, DIN:]
    z = np.zeros((DIN, 64), F32)
    w1blob = np.concatenate(
        [W1A.T, z, z, W1A.T, W1B.T, W1B.T], axis=1
    ).astype(BF16)
    z64 = np.zeros((64, 128), F32)
    wblob = np.concatenate(
        [
            np.concatenate([W2.T, z64], axis=0),
            np.concatenate([z64, W2.T], axis=0),
            W3.T[:, 0:128],
            W3.T[:, 128:256],
        ],
        axis=1,
    ).astype(BF16)
    # v1t: V1.T is [256, 512] -> k-tiles stacked on cols [128, 2, 512]
    v1t_cols = V1.T.reshape(2, 128, 512).transpose(1, 0, 2).reshape(128, 1024)
    vblob = np.concatenate(
        [v1t_cols,
         V2.T.reshape(4, 128, 256).transpose(1, 0, 2).reshape(128, 1024),
         V3.T.reshape(2, 128, 40).transpose(1, 0, 2).reshape(128, 80)],
        axis=1,
    ).astype(BF16)
    cblob = np.zeros((128, 11), F32)
    cblob[:, 0] = np.concatenate([b1, b1])
    cblob[:, 1] = b2
    cblob[:, 2:4] = b3.reshape(2, 128).T
    cblob[:, 4:8] = c1.reshape(4, 128).T
    cblob[:, 8:10] = c2.reshape(2, 128).T
    cblob[0:40, 10] = c3

    shared = dict(wblob=wblob, w1blob=w1blob, vblob=vblob, cblob=cblob)

    Xv = X.reshape(B, D, M, DIN)
    in_maps = []
    for c in range(N_CORES):
        xts = np.empty((GROUPS_PER_CORE, DIN, M), F32)
        for gi in range(GROUPS_PER_CORE):
            g = 2 * c + gi
            bb, dd = g // D, g % D
            xts[gi] = Xv[bb, dd].T
        in_maps.append(dict(shared, xt=xts.astype(BF16)))
    return in_maps


def _run(inputs, trace=False):
    nc = _get_program()
    in_maps = _make_in_maps(inputs)
    res = run_bass_kernel_spmd(nc, in_maps, list(range(N_CORES)), trace=trace)
    ys = np.stack([res.results[c]["y"] for c in range(N_CORES)])  # [8, 2, 40]
    y16 = ys.reshape(B, D, 40)
    out = y16.max(axis=1).astype(F32)
    return out, res


def kernel(**inputs):
    out, _ = _run(inputs, trace=False)
    return out


# revision 10
# speedup vs baseline: 1.0052x; 1.0052x over previous
"""Trainium2 Bass kernel for BoostedPointPairNet2.

Model (per (b, d) group, m = 128 points, din = 3):
  H1(i,j) = relu(W1A @ x_j + W1B @ x_i + b1)          (64)
  H2(i,j) = relu(W2 @ H1 + b2)                        (128)
  G(i,j)  = W3 @ H2                                    (256, b3 deferred)
  P       = max_{i,j} G + b3                           (256)
  Y       = V3 @ relu(V2 @ relu(V1 @ P + c1) + c2) + c3  (40)
  out[b]  = max_d Y[b, d]

Sharding: 16 (b, d) groups over 8 cores, 2 groups per core. Weights
replicated. Each core returns its two groups' Y rows; the host does the
final max over d (the trivial "all-gather" of a (b, 40) output).

Per-core dataflow ("stacked pairs" layout): channels of two j-values are
stacked on the 128 SBUF partitions (j even -> partitions 0-63, j odd ->
64-127), so layer-1 (only 64 channels) runs at full partition width as ONE
fused DVE tensor_scalar op per j-pair: relu(v_stacked + (u_j + b1)).
L2 unstacks via block-padded weights [W2^T;0] / [0;W2^T]; L3 streams H2
through W3^T halves into PSUM.

The G max-reduction is the bottleneck: PSUM can only be read by the DVE
(0.96 GHz, the only engine with max) and ACT (1.2 GHz, no max). So G
tiles alternate between (a) direct DVE reduce_max from PSUM and (b) ACT
copy/cast to bf16 SBUF + DVE running tensor_tensor max at the 2x bf16
mode - balancing the two engines. Matmuls and big activations run in
bf16 (fp32 PSUM accumulation); the tiny F-MLP runs in fp32.
"""

import numpy as np
import ml_dtypes

import bass_rust
import concourse.bass as bass
import concourse.mybir as mybir
from concourse.tile import TileContext
from concourse.bass_utils import run_bass_kernel_spmd

BF16 = ml_dtypes.bfloat16
F32 = np.float32
DT = mybir.dt
ALU = mybir.AluOpType
AX = mybir.AxisListType
RELU = mybir.ActivationFunctionType.Relu

N_CORES = 8
B, N, DIN = 4, 512, 3
D = 4                    # boost factor
M = N // D               # 128 points per group
GROUPS_PER_CORE = 2
JP = M // 2              # 64 stacked j-pairs per group
NPAIR = JP // 4          # 16 pipeline iterations per group (4 jp each)
NGT = 2 * NPAIR          # 32 G-psum tiles per group
# G tiles with (t % DIRECT_MOD == 0) are reduced directly from PSUM by the
# DVE; the rest are ACT-copied to bf16 SBUF and running-maxed by the DVE.
DIRECT_MOD = 3
N_DIRECT = (NGT + DIRECT_MOD - 1) // DIRECT_MOD


# ---------------------------------------------------------------------------
# Workaround: this walrus build accepts at most ONE sync wait per instruction
# ("Too many sync wait commands"), while Tile emits up to 3. Hoist extra
# waits onto same-engine nop instructions inserted just before the offender
# (engines execute their queue in order, so the AND-wait semantics hold).
# ---------------------------------------------------------------------------
def _split_multi_waits(nc):
    seq = 0
    for fn in nc.m.functions:
        for bb in fn.blocks:
            new = []
            changed = False
            for ins in bb.instructions:
                si = ins.sync_info
                waits = list(si.on_wait) if si is not None and si.on_wait else []
                if len(waits) > 1:
                    changed = True
                    for w in waits[:-1]:
                        seq += 1
                        new.append(
                            mybir.InstNoOp(
                                name=f"I-wsplit-{seq}",
                                engine=ins.engine,
                                sync_info=bass_rust.SyncInfo(
                                    on_wait=[w], on_update=[]
                                ),
                            )
                        )
                    ins.sync_info = bass_rust.SyncInfo(
                        on_wait=[waits[-1]], on_update=list(si.on_update or [])
                    )
                new.append(ins)
            if changed:
                bb.instructions = new


# ---------------------------------------------------------------------------
# Device program
# ---------------------------------------------------------------------------
def _build_program():
    nc = bass.Bass(
        "TRN2", target_bir_lowering=False, debug=False, num_devices=N_CORES
    )

    xt = nc.declare_dram_parameter(
        "xt", [GROUPS_PER_CORE, DIN, M], DT.bfloat16, isOutput=False
    )
    # col 0:128 w2a=[W2T;0], 128:256 w2b=[0;W2T], 256:384 w3a, 384:512 w3b
    wblob = nc.declare_dram_parameter("wblob", [128, 512], DT.bfloat16, isOutput=False)
    # col 0:128 w1a_even=[W1AT|0], 128:256 w1a_odd=[0|W1AT], 256:384 w1b2=[W1BT|W1BT]
    w1blob = nc.declare_dram_parameter("w1blob", [DIN, 384], DT.bfloat16, isOutput=False)
    # v1t (2x512) | v2t (4x256) | v3t (2x40)
    vblob = nc.declare_dram_parameter("vblob", [128, 2128], DT.bfloat16, isOutput=False)
    # col 0 b1st, 1 b2c, 2:4 b3_2, 4:8 c1_4, 8:10 c2_2, 10 c3 (rows 0:40)
    cblob = nc.declare_dram_parameter("cblob", [128, 11], DT.float32, isOutput=False)
    y_out = nc.declare_dram_parameter(
        "y", [GROUPS_PER_CORE, 40], DT.float32, isOutput=True
    )

    with TileContext(nc) as tc:
        with (
            tc.tile_pool(name="singles", bufs=1) as singles,
            tc.tile_pool(name="xtp", bufs=2) as xtp,
            tc.tile_pool(name="v2p", bufs=2) as v2pool,
            tc.tile_pool(name="uup", bufs=2) as uupool,
            tc.tile_pool(name="h1p", bufs=3) as h1pool,
            tc.tile_pool(name="h2p", bufs=3) as h2pool,
            tc.tile_pool(name="gcp", bufs=4) as gcpool,
            tc.tile_pool(name="raccp", bufs=2) as raccpool,
            tc.tile_pool(name="fmlp", bufs=8) as fmlp,
            tc.tile_pool(name="psum", bufs=2, space="PSUM") as psum,
        ):
            # ---- load inputs; issue order = need order (2 HWDGE queues) ----
            sb_xts = []
            for g in range(GROUPS_PER_CORE):
                t = xtp.tile([DIN, M], DT.bfloat16)
                sb_xts.append(t)
            nc.sync.dma_start(out=sb_xts[0], in_=xt[0])
            sb_w1 = singles.tile([DIN, 384], DT.bfloat16, tag="w1blob")
            nc.scalar.dma_start(out=sb_w1, in_=w1blob[:, :])
            sb_c = singles.tile([128, 11], DT.float32, tag="cblob")
            nc.sync.dma_start(out=sb_c, in_=cblob[:, :])
            sb_w = singles.tile([128, 512], DT.bfloat16, tag="wblob")
            nc.scalar.dma_start(out=sb_w, in_=wblob[:, :])
            sb_v = singles.tile([128, 2128], DT.bfloat16, tag="vblob")
            nc.sync.dma_start(out=sb_v, in_=vblob[:, :])
            nc.scalar.dma_start(out=sb_xts[1], in_=xt[1])

            sb_w2a, sb_w2b = sb_w[:, 0:128], sb_w[:, 128:256]
            sb_w3a, sb_w3b = sb_w[:, 256:384], sb_w[:, 384:512]
            sb_w1a_e, sb_w1a_o = sb_w1[:, 0:128], sb_w1[:, 128:256]
            sb_w1b2 = sb_w1[:, 256:384]
            sb_b1st = sb_c[:, 0:1]
            sb_b2c = sb_c[:, 1:2]
            sb_b3_2 = sb_c[:, 2:4]
            sb_c1_4 = sb_c[:, 4:8]
            sb_c2_2 = sb_c[:, 8:10]
            sb_c3c = sb_c[0:40, 10:11]

            def v1t(k):  # [128, 512] f32, k in 0..1
                return sb_v[:, 512 * k : 512 * (k + 1)]

            def v2t(k):  # [128, 256] f32, k in 0..3
                return sb_v[:, 1024 + 256 * k : 1024 + 256 * (k + 1)]

            def v3t(k):  # [128, 40] f32, k in 0..1
                return sb_v[:, 2048 + 40 * k : 2048 + 40 * (k + 1)]

            for g in range(GROUPS_PER_CORE):
                # ---- per-group prep: stacked v (V2) and u+b1 (UU) ----
                sb_xt = sb_xts[g]
                xt_eo = sb_xt.rearrange("k (j two) -> k two j", two=2)

                v2ps = psum.tile([128, M], DT.float32, tag="l2")
                nc.tensor.matmul(v2ps, lhsT=sb_w1b2, rhs=sb_xt, start=True, stop=True)
                uups = psum.tile([128, JP], DT.float32, tag="g")
                nc.tensor.matmul(
                    uups, lhsT=sb_w1a_e, rhs=xt_eo[:, 0, :], start=True, stop=False
                )
                nc.tensor.matmul(
                    uups, lhsT=sb_w1a_o, rhs=xt_eo[:, 1, :], start=False, stop=True
                )
                sb_v2 = v2pool.tile([128, M], DT.bfloat16)
                nc.vector.tensor_copy(out=sb_v2, in_=v2ps)
                sb_uu = uupool.tile([128, JP], DT.float32)
                nc.vector.tensor_scalar_add(out=sb_uu, in0=uups, scalar1=sb_b1st)

                racc = raccpool.tile([128, 2, N_DIRECT], DT.float32)
                rbs = []
                for h in range(2):
                    rb = raccpool.tile([128, 1024], DT.bfloat16, tag=f"rb{h}")
                    nc.gpsimd.memset(rb, -1e30)
                    rbs.append(rb)

                # ---- main pairwise pipeline: 4 j-pairs per iteration ----
                for it in range(NPAIR):
                    h1 = h1pool.tile([128, 512], DT.bfloat16)
                    for jj in range(4):
                        jp = it * 4 + jj
                        nc.vector.tensor_scalar(
                            out=h1[:, jj * M : (jj + 1) * M],
                            in0=sb_v2,
                            scalar1=sb_uu[:, jp : jp + 1],
                            scalar2=0.0,
                            op0=ALU.add,
                            op1=ALU.max,
                        )
                    # L2: weight-grouped matmuls into one 2-bank psum tile
                    l2ps = psum.tile([128, 1024], DT.float32, tag="l2")
                    nc.tensor.matmul(
                        l2ps[:, 0:256], lhsT=sb_w2a, rhs=h1[:, 0:256],
                        start=True, stop=True,
                    )
                    nc.tensor.matmul(
                        l2ps[:, 512:768], lhsT=sb_w2a, rhs=h1[:, 256:512],
                        start=True, stop=True,
                    )
                    nc.tensor.matmul(
                        l2ps[:, 256:512], lhsT=sb_w2b, rhs=h1[:, 0:256],
                        start=True, stop=True,
                    )
                    nc.tensor.matmul(
                        l2ps[:, 768:1024], lhsT=sb_w2b, rhs=h1[:, 256:512],
                        start=True, stop=True,
                    )
                    h2 = h2pool.tile([128, 1024], DT.bfloat16)
                    nc.scalar.activation(
                        out=h2, in_=l2ps, func=RELU, bias=sb_b2c, scale=1.0
                    )
                    # L3: weight-grouped into two G tiles (1024 pairs total)
                    gpa = psum.tile([128, 2, 512], DT.float32, tag="g")
                    gpb = psum.tile([128, 2, 512], DT.float32, tag="g")
                    nc.tensor.matmul(
                        gpa[:, 0, :], lhsT=sb_w3a, rhs=h2[:, 0:512],
                        start=True, stop=True,
                    )
                    nc.tensor.matmul(
                        gpb[:, 0, :], lhsT=sb_w3a, rhs=h2[:, 512:1024],
                        start=True, stop=True,
                    )
                    nc.tensor.matmul(
                        gpa[:, 1, :], lhsT=sb_w3b, rhs=h2[:, 0:512],
                        start=True, stop=True,
                    )
                    nc.tensor.matmul(
                        gpb[:, 1, :], lhsT=sb_w3b, rhs=h2[:, 512:1024],
                        start=True, stop=True,
                    )
                    for half, gp in enumerate((gpa, gpb)):
                        t = it * 2 + half
                        if t % DIRECT_MOD == 0:
                            nc.vector.reduce_max(
                                out=racc[:, :, t // DIRECT_MOD : t // DIRECT_MOD + 1],
                                in_=gp, axis=AX.X,
                            )
                        else:
                            gc = gcpool.tile([128, 1024], DT.bfloat16)
                            nc.scalar.copy(
                                out=gc, in_=gp.rearrange("p a b -> p (a b)")
                            )
                            rb = rbs[t % 2]
                            nc.vector.tensor_tensor(
                                out=rb, in0=gc, in1=rb, op=ALU.max
                            )

                # ---- P = max over accumulators, + b3; F MLP (fp32, N=1) ----
                pm1 = fmlp.tile([128, 2], DT.float32, tag="pm1")
                nc.vector.reduce_max(out=pm1, in_=racc, axis=AX.X)
                nc.vector.tensor_tensor(
                    out=rbs[0], in0=rbs[0], in1=rbs[1], op=ALU.max
                )
                pm2 = fmlp.tile([128, 2], DT.float32, tag="pm2")
                nc.vector.reduce_max(
                    out=pm2, in_=rbs[0].rearrange("p (a b) -> p a b", a=2), axis=AX.X
                )
                pmx = fmlp.tile([128, 2], DT.float32, tag="pmx")
                nc.vector.tensor_tensor(out=pmx, in0=pm1, in1=pm2, op=ALU.max)
                pb = fmlp.tile([128, 2], DT.bfloat16, tag="pb")
                nc.vector.tensor_tensor(out=pb, in0=pmx, in1=sb_b3_2, op=ALU.add)

                y1ps = psum.tile([128, 4], DT.float32, tag="l2")
                for mm in range(4):
                    for kk in range(2):
                        nc.tensor.matmul(
                            y1ps[:, mm : mm + 1],
                            lhsT=v1t(kk)[:, mm * 128 : (mm + 1) * 128],
                            rhs=pb[:, kk : kk + 1],
                            start=(kk == 0),
                            stop=(kk == 1),
                        )
                y1pre = fmlp.tile([128, 4], DT.float32, tag="y1pre")
                nc.vector.tensor_tensor(out=y1pre, in0=y1ps, in1=sb_c1_4, op=ALU.add)
                y1 = fmlp.tile([128, 4], DT.bfloat16, tag="y1")
                nc.vector.tensor_scalar_max(out=y1, in0=y1pre, scalar1=0.0)

                y2ps = psum.tile([128, 2], DT.float32, tag="l2")
                for mm in range(2):
                    for kk in range(4):
                        nc.tensor.matmul(
                            y2ps[:, mm : mm + 1],
                            lhsT=v2t(kk)[:, mm * 128 : (mm + 1) * 128],
                            rhs=y1[:, kk : kk + 1],
                            start=(kk == 0),
                            stop=(kk == 3),
                        )
                y2pre = fmlp.tile([128, 2], DT.float32, tag="y2pre")
                nc.vector.tensor_tensor(out=y2pre, in0=y2ps, in1=sb_c2_2, op=ALU.add)
                y2 = fmlp.tile([128, 2], DT.bfloat16, tag="y2")
                nc.vector.tensor_scalar_max(out=y2, in0=y2pre, scalar1=0.0)

                y3ps = psum.tile([40, 1], DT.float32, tag="g")
                for kk in range(2):
                    nc.tensor.matmul(
                        y3ps,
                        lhsT=v3t(kk)[:, 0:40],
                        rhs=y2[:, kk : kk + 1],
                        start=(kk == 0),
                        stop=(kk == 1),
                    )
                y3 = fmlp.tile([40, 1], DT.float32, tag="y3")
                nc.vector.tensor_scalar_add(out=y3, in0=y3ps, scalar1=sb_c3c)
                nc.sync.dma_start(out=y_out[g, :], in_=y3)

    _split_multi_waits(nc)
    return nc


# ---------------------------------------------------------------------------
# Host side
# ---------------------------------------------------------------------------
_NC_CACHE = None


def _get_program():
    global _NC_CACHE
    if _NC_CACHE is None:
        _NC_CACHE = _build_program()
    return _NC_CACHE


def _make_in_maps(inputs):
    X = np.asarray(inputs["X"], F32)
    W1 = np.asarray(inputs["W1"], F32)
    b1 = np.asarray(inputs["b1"], F32)
    W2 = np.asarray(inputs["W2"], F32)
    b2 = np.asarray(inputs["b2"], F32)
    W3 = np.asarray(inputs["W3"], F32)
    b3 = np.asarray(inputs["b3"], F32)
    V1 = np.asarray(inputs["V1"], F32)
    c1 = np.asarray(inputs["c1"], F32)
    V2 = np.asarray(inputs["V2"], F32)
    c2 = np.asarray(inputs["c2"], F32)
    V3 = np.asarray(inputs["V3"], F32)
    c3 = np.asarray(inputs["c3"], F32)

    W1A, W1B = W1[:, :DIN], W1[:, DIN:]
    z = np.zeros((DIN, 64), F32)
    w1blob = np.concatenate(
        [W1A.T, z, z, W1A.T, W1B.T, W1B.T], axis=1
    ).astype(BF16)
    z64 = np.zeros((64, 128), F32)
    wblob = np.concatenate(
        [
            np.concatenate([W2.T, z64], axis=0),
            np.concatenate([z64, W2.T], axis=0),
            W3.T[:, 0:128],
            W3.T[:, 128:256],
        ],
        axis=1,
    ).astype(BF16)
    # v1t: V1.T is [256, 512] -> k-tiles stacked on cols [128, 2, 512]
    v1t_cols = V1.T.reshape(2, 128, 512).transpose(1, 0, 2).reshape(128, 1024)
    vblob = np.concatenate(
        [v1t_cols,
         V2.T.reshape(4, 128, 256).transpose(1, 0, 2).reshape(128, 1024),
         V3.T.reshape(2, 128, 40).transpose(1, 0, 2).reshape(128, 80)],
        axis=1,
    ).astype(BF16)
    cblob = np.zeros((128, 11), F32)
    cblob[:, 0] = np.concatenate([b1, b1])
    cblob[:, 1] = b2
    cblob[:, 2:4] = b3.reshape(2, 128).T
    cblob[:, 4:8] = c1.reshape(4, 128).T
    cblob[:, 8:10] = c2.reshape(2, 128).T
    cblob[0:40, 10] = c3

    shared = dict(wblob=wblob, w1blob=w1blob, vblob=vblob, cblob=cblob)

    Xv = X.reshape(B, D, M, DIN)
    in_maps = []
    for c in range(N_CORES):
        xts = np.empty((GROUPS_PER_CORE, DIN, M), F32)
        for gi in range(GROUPS_PER_CORE):
            g = 2 * c + gi
            bb, dd = g // D, g % D
            xts[gi] = Xv[bb, dd].T
        in_maps.append(dict(shared, xt=xts.astype(BF16)))
    return in_maps


def _run(inputs, trace=False):
    nc = _get_program()
    in_maps = _make_in_maps(inputs)
    res = run_bass_kernel_spmd(nc, in_maps, list(range(N_CORES)), trace=trace)
    ys = np.stack([res.results[c]["y"] for c in range(N_CORES)])  # [8, 2, 40]
    y16 = ys.reshape(B, D, 40)
    out = y16.max(axis=1).astype(F32)
    return out, res


def kernel(**inputs):
    out, _ = _run(inputs, trace=False)
    return out


# revision 12
# speedup vs baseline: 1.0158x; 1.0105x over previous
"""Trainium2 Bass kernel for BoostedPointPairNet2.

Model (per (b, d) group, m = 128 points, din = 3):
  H1(i,j) = relu(W1A @ x_j + W1B @ x_i + b1)          (64)
  H2(i,j) = relu(W2 @ H1 + b2)                        (128)
  G(i,j)  = W3 @ H2                                    (256, b3 deferred)
  P       = max_{i,j} G + b3                           (256)
  Y       = V3 @ relu(V2 @ relu(V1 @ P + c1) + c2) + c3  (40)
  out[b]  = max_d Y[b, d]

Sharding: 16 (b, d) groups over 8 cores, 2 groups per core. Weights
replicated. Each core returns its two groups' Y rows; the host does the
final max over d (the trivial "all-gather" of a (b, 40) output).

Per-core dataflow ("stacked pairs" layout): channels of two j-values are
stacked on the 128 SBUF partitions (j even -> partitions 0-63, j odd ->
64-127), so layer-1 (only 64 channels) runs at full partition width as ONE
fused DVE tensor_scalar op per j-pair: relu(v_stacked + (u_j + b1)).
L2 unstacks via block-padded weights [W2^T;0] / [0;W2^T]; L3 streams H2
through W3^T halves into PSUM.

The G max-reduction is the bottleneck: PSUM can only be read by the DVE
(0.96 GHz, the only engine with max) and ACT (1.2 GHz, no max). So G
tiles alternate between (a) direct DVE reduce_max from PSUM and (b) ACT
copy/cast to bf16 SBUF + DVE running tensor_tensor max at the 2x bf16
mode - balancing the two engines. Matmuls and big activations run in
bf16 (fp32 PSUM accumulation); the tiny F-MLP runs in fp32.
"""

import numpy as np
import ml_dtypes

import bass_rust
import concourse.bass as bass
import concourse.mybir as mybir
from concourse.tile import TileContext
from concourse.bass_utils import run_bass_kernel_spmd

BF16 = ml_dtypes.bfloat16
F32 = np.float32
DT = mybir.dt
ALU = mybir.AluOpType
AX = mybir.AxisListType
RELU = mybir.ActivationFunctionType.Relu

N_CORES = 8
B, N, DIN = 4, 512, 3
D = 4                    # boost factor
M = N // D               # 128 points per group
GROUPS_PER_CORE = 2
JP = M // 2              # 64 stacked j-pairs per group
NPAIR = JP // 4          # 16 pipeline iterations per group (4 jp each)
NGT = 2 * NPAIR          # 32 G-psum tiles per group
# G tiles with (t % DIRECT_MOD == 0) are reduced directly from PSUM by the
# DVE; the rest are ACT-copied to bf16 SBUF and running-maxed by the DVE.
DIRECT_MOD = 3
N_DIRECT = (NGT + DIRECT_MOD - 1) // DIRECT_MOD


# ---------------------------------------------------------------------------
# Workaround: this walrus build accepts at most ONE sync wait per instruction
# ("Too many sync wait commands"), while Tile emits up to 3. Hoist extra
# waits onto same-engine nop instructions inserted just before the offender
# (engines execute their queue in order, so the AND-wait semantics hold).
# ---------------------------------------------------------------------------
def _split_multi_waits(nc):
    seq = 0
    for fn in nc.m.functions:
        for bb in fn.blocks:
            new = []
            changed = False
            for ins in bb.instructions:
                si = ins.sync_info
                waits = list(si.on_wait) if si is not None and si.on_wait else []
                if len(waits) > 1:
                    changed = True
                    for w in waits[:-1]:
                        seq += 1
                        new.append(
                            mybir.InstNoOp(
                                name=f"I-wsplit-{seq}",
                                engine=ins.engine,
                                sync_info=bass_rust.SyncInfo(
                                    on_wait=[w], on_update=[]
                                ),
                            )
                        )
                    ins.sync_info = bass_rust.SyncInfo(
                        on_wait=[waits[-1]], on_update=list(si.on_update or [])
                    )
                new.append(ins)
            if changed:
                bb.instructions = new


# ---------------------------------------------------------------------------
# Device program
# ---------------------------------------------------------------------------
def _build_program():
    nc = bass.Bass(
        "TRN2", target_bir_lowering=False, debug=False, num_devices=N_CORES
    )

    xt = nc.declare_dram_parameter(
        "xt", [GROUPS_PER_CORE, DIN, M], DT.bfloat16, isOutput=False
    )
    # col 0:128 w2a=[W2T;0], 128:256 w2b=[0;W2T], 256:384 w3a, 384:512 w3b
    wblob = nc.declare_dram_parameter("wblob", [128, 512], DT.bfloat16, isOutput=False)
    # col 0:128 w1a_even=[W1AT|0], 128:256 w1a_odd=[0|W1AT], 256:384 w1b2=[W1BT|W1BT]
    w1blob = nc.declare_dram_parameter("w1blob", [DIN, 384], DT.bfloat16, isOutput=False)
    # v1t (2x512) | v2t (4x256) | v3t (2x40)
    vblob = nc.declare_dram_parameter("vblob", [128, 2128], DT.bfloat16, isOutput=False)
    # col 0 b1st, 1 b2c, 2:4 b3_2, 4:8 c1_4, 8:10 c2_2, 10 c3 (rows 0:40)
    cblob = nc.declare_dram_parameter("cblob", [128, 11], DT.float32, isOutput=False)
    y_out = nc.declare_dram_parameter(
        "y", [GROUPS_PER_CORE, 40], DT.float32, isOutput=True
    )

    with TileContext(nc) as tc:
        with (
            tc.tile_pool(name="singles", bufs=1) as singles,
            tc.tile_pool(name="xtp", bufs=2) as xtp,
            tc.tile_pool(name="v2p", bufs=2) as v2pool,
            tc.tile_pool(name="uup", bufs=2) as uupool,
            tc.tile_pool(name="h1p", bufs=3) as h1pool,
            tc.tile_pool(name="h2p", bufs=3) as h2pool,
            tc.tile_pool(name="gcp", bufs=4) as gcpool,
            tc.tile_pool(name="raccp", bufs=2) as raccpool,
            tc.tile_pool(name="fmlp", bufs=8) as fmlp,
            tc.tile_pool(name="psum", bufs=2, space="PSUM") as psum,
        ):
            # ---- load inputs; issue order = need order (2 HWDGE queues) ----
            sb_xts = []
            for g in range(GROUPS_PER_CORE):
                t = xtp.tile([DIN, M], DT.bfloat16)
                sb_xts.append(t)
            nc.sync.dma_start(out=sb_xts[0], in_=xt[0])
            sb_w1 = singles.tile([DIN, 384], DT.bfloat16, tag="w1blob")
            nc.scalar.dma_start(out=sb_w1, in_=w1blob[:, :])
            sb_c = singles.tile([128, 11], DT.float32, tag="cblob")
            nc.sync.dma_start(out=sb_c, in_=cblob[:, :])
            sb_w = singles.tile([128, 512], DT.bfloat16, tag="wblob")
            nc.scalar.dma_start(out=sb_w, in_=wblob[:, :])
            sb_v = singles.tile([128, 2128], DT.bfloat16, tag="vblob")
            nc.sync.dma_start(out=sb_v, in_=vblob[:, :])
            nc.scalar.dma_start(out=sb_xts[1], in_=xt[1])

            sb_w2a, sb_w2b = sb_w[:, 0:128], sb_w[:, 128:256]
            sb_w3a, sb_w3b = sb_w[:, 256:384], sb_w[:, 384:512]
            sb_w1a_e, sb_w1a_o = sb_w1[:, 0:128], sb_w1[:, 128:256]
            sb_w1b2 = sb_w1[:, 256:384]
            sb_b1st = sb_c[:, 0:1]
            sb_b2c = sb_c[:, 1:2]
            sb_b3_2 = sb_c[:, 2:4]
            sb_c1_4 = sb_c[:, 4:8]
            sb_c2_2 = sb_c[:, 8:10]
            sb_c3c = sb_c[0:40, 10:11]

            def v1t(k):  # [128, 512] f32, k in 0..1
                return sb_v[:, 512 * k : 512 * (k + 1)]

            def v2t(k):  # [128, 256] f32, k in 0..3
                return sb_v[:, 1024 + 256 * k : 1024 + 256 * (k + 1)]

            def v3t(k):  # [128, 40] f32, k in 0..1
                return sb_v[:, 2048 + 40 * k : 2048 + 40 * (k + 1)]

            for g in range(GROUPS_PER_CORE):
                # ---- per-group prep: stacked v (V2) and u+b1 (UU) ----
                sb_xt = sb_xts[g]
                xt_eo = sb_xt.rearrange("k (j two) -> k two j", two=2)

                v2ps = psum.tile([128, M], DT.float32, tag="l2")
                nc.tensor.matmul(v2ps, lhsT=sb_w1b2, rhs=sb_xt, start=True, stop=True)
                uups = psum.tile([128, JP], DT.float32, tag="g")
                nc.tensor.matmul(
                    uups, lhsT=sb_w1a_e, rhs=xt_eo[:, 0, :], start=True, stop=False
                )
                nc.tensor.matmul(
                    uups, lhsT=sb_w1a_o, rhs=xt_eo[:, 1, :], start=False, stop=True
                )
                sb_v2 = v2pool.tile([128, M], DT.bfloat16)
                nc.vector.tensor_copy(out=sb_v2, in_=v2ps)
                sb_uu = uupool.tile([128, JP], DT.float32)
                nc.vector.tensor_scalar_add(out=sb_uu, in0=uups, scalar1=sb_b1st)

                racc = raccpool.tile([128, 2, N_DIRECT], DT.float32)
                rbs = []
                rb_init = [False, False]
                for h in range(2):
                    rb = raccpool.tile([128, 1024], DT.bfloat16, tag=f"rb{h}")
                    rbs.append(rb)

                # ---- main pairwise pipeline: 4 j-pairs per iteration ----
                for it in range(NPAIR):
                    h1 = h1pool.tile([128, 512], DT.bfloat16)
                    for jj in range(4):
                        jp = it * 4 + jj
                        nc.vector.tensor_scalar(
                            out=h1[:, jj * M : (jj + 1) * M],
                            in0=sb_v2,
                            scalar1=sb_uu[:, jp : jp + 1],
                            scalar2=0.0,
                            op0=ALU.add,
                            op1=ALU.max,
                        )
                    # L2: weight-grouped matmuls into one 2-bank psum tile
                    l2ps = psum.tile([128, 1024], DT.float32, tag="l2")
                    nc.tensor.matmul(
                        l2ps[:, 0:256], lhsT=sb_w2a, rhs=h1[:, 0:256],
                        start=True, stop=True,
                    )
                    nc.tensor.matmul(
                        l2ps[:, 512:768], lhsT=sb_w2a, rhs=h1[:, 256:512],
                        start=True, stop=True,
                    )
                    nc.tensor.matmul(
                        l2ps[:, 256:512], lhsT=sb_w2b, rhs=h1[:, 0:256],
                        start=True, stop=True,
                    )
                    nc.tensor.matmul(
                        l2ps[:, 768:1024], lhsT=sb_w2b, rhs=h1[:, 256:512],
                        start=True, stop=True,
                    )
                    h2 = h2pool.tile([128, 1024], DT.bfloat16)
                    nc.scalar.activation(
                        out=h2, in_=l2ps, func=RELU, bias=sb_b2c, scale=1.0
                    )
                    # L3: weight-grouped into two G tiles (1024 pairs total)
                    gpa = psum.tile([128, 2, 512], DT.float32, tag="g")
                    gpb = psum.tile([128, 2, 512], DT.float32, tag="g")
                    nc.tensor.matmul(
                        gpa[:, 0, :], lhsT=sb_w3a, rhs=h2[:, 0:512],
                        start=True, stop=True,
                    )
                    nc.tensor.matmul(
                        gpb[:, 0, :], lhsT=sb_w3a, rhs=h2[:, 512:1024],
                        start=True, stop=True,
                    )
                    nc.tensor.matmul(
                        gpa[:, 1, :], lhsT=sb_w3b, rhs=h2[:, 0:512],
                        start=True, stop=True,
                    )
                    nc.tensor.matmul(
                        gpb[:, 1, :], lhsT=sb_w3b, rhs=h2[:, 512:1024],
                        start=True, stop=True,
                    )
                    for half, gp in enumerate((gpa, gpb)):
                        t = it * 2 + half
                        if t % DIRECT_MOD == 0:
                            nc.vector.reduce_max(
                                out=racc[:, :, t // DIRECT_MOD : t // DIRECT_MOD + 1],
                                in_=gp, axis=AX.X,
                            )
                        else:
                            gc = gcpool.tile([128, 1024], DT.bfloat16)
                            nc.scalar.copy(
                                out=gc, in_=gp.rearrange("p a b -> p (a b)")
                            )
                            rb = rbs[t % 2]
                            if not rb_init[t % 2]:
                                rb_init[t % 2] = True
                                nc.vector.tensor_copy(out=rb, in_=gc)
                            else:
                                nc.vector.tensor_tensor(
                                    out=rb, in0=gc, in1=rb, op=ALU.max
                                )

                # ---- P = max over accumulators, + b3; F MLP (fp32, N=1) ----
                pm1 = fmlp.tile([128, 2], DT.float32, tag="pm1")
                nc.vector.reduce_max(out=pm1, in_=racc, axis=AX.X)
                nc.vector.tensor_tensor(
                    out=rbs[0], in0=rbs[0], in1=rbs[1], op=ALU.max
                )
                pm2 = fmlp.tile([128, 2], DT.float32, tag="pm2")
                nc.vector.reduce_max(
                    out=pm2, in_=rbs[0].rearrange("p (a b) -> p a b", a=2), axis=AX.X
                )
                pmx = fmlp.tile([128, 2], DT.float32, tag="pmx")
                nc.vector.tensor_tensor(out=pmx, in0=pm1, in1=pm2, op=ALU.max)
                pb = fmlp.tile([128, 2], DT.bfloat16, tag="pb")
                nc.vector.tensor_tensor(out=pb, in0=pmx, in1=sb_b3_2, op=ALU.add)

                y1ps = psum.tile([128, 4], DT.float32, tag="l2")
                for mm in range(4):
                    for kk in range(2):
                        nc.tensor.matmul(
                            y1ps[:, mm : mm + 1],
                            lhsT=v1t(kk)[:, mm * 128 : (mm + 1) * 128],
                            rhs=pb[:, kk : kk + 1],
                            start=(kk == 0),
                            stop=(kk == 1),
                        )
                y1pre = fmlp.tile([128, 4], DT.float32, tag="y1pre")
                nc.vector.tensor_tensor(out=y1pre, in0=y1ps, in1=sb_c1_4, op=ALU.add)
                y1 = fmlp.tile([128, 4], DT.bfloat16, tag="y1")
                nc.vector.tensor_scalar_max(out=y1, in0=y1pre, scalar1=0.0)

                y2ps = psum.tile([128, 2], DT.float32, tag="l2")
                for mm in range(2):
                    for kk in range(4):
                        nc.tensor.matmul(
                            y2ps[:, mm : mm + 1],
                            lhsT=v2t(kk)[:, mm * 128 : (mm + 1) * 128],
                            rhs=y1[:, kk : kk + 1],
                            start=(kk == 0),
                            stop=(kk == 3),
                        )
                y2pre = fmlp.tile([128, 2], DT.float32, tag="y2pre")
                nc.vector.tensor_tensor(out=y2pre, in0=y2ps, in1=sb_c2_2, op=ALU.add)
                y2 = fmlp.tile([128, 2], DT.bfloat16, tag="y2")
                nc.vector.tensor_scalar_max(out=y2, in0=y2pre, scalar1=0.0)

                y3ps = psum.tile([40, 1], DT.float32, tag="g")
                for kk in range(2):
                    nc.tensor.matmul(
                        y3ps,
                        lhsT=v3t(kk)[:, 0:40],
                        rhs=y2[:, kk : kk + 1],
                        start=(kk == 0),
                        stop=(kk == 1),
                    )
                y3 = fmlp.tile([40, 1], DT.float32, tag="y3")
                nc.vector.tensor_scalar_add(out=y3, in0=y3ps, scalar1=sb_c3c)
                nc.sync.dma_start(out=y_out[g, :], in_=y3)

    _split_multi_waits(nc)
    return nc


# ---------------------------------------------------------------------------
# Host side
# ---------------------------------------------------------------------------
_NC_CACHE = None


def _get_program():
    global _NC_CACHE
    if _NC_CACHE is None:
        _NC_CACHE = _build_program()
    return _NC_CACHE


def _make_in_maps(inputs):
    X = np.asarray(inputs["X"], F32)
    W1 = np.asarray(inputs["W1"], F32)
    b1 = np.asarray(inputs["b1"], F32)
    W2 = np.asarray(inputs["W2"], F32)
    b2 = np.asarray(inputs["b2"], F32)
    W3 = np.asarray(inputs["W3"], F32)
    b3 = np.asarray(inputs["b3"], F32)
    V1 = np.asarray(inputs["V1"], F32)
    c1 = np.asarray(inputs["c1"], F32)
    V2 = np.asarray(inputs["V2"], F32)
    c2 = np.asarray(inputs["c2"], F32)
    V3 = np.asarray(inputs["V3"], F32)
    c3 = np.asarray(inputs["c3"], F32)

    W1A, W1B = W1[:, :DIN], W1[:, DIN:]
    z = np.zeros((DIN, 64), F32)
    w1blob = np.concatenate(
        [W1A.T, z, z, W1A.T, W1B.T, W1B.T], axis=1
    ).astype(BF16)
    z64 = np.zeros((64, 128), F32)
    wblob = np.concatenate(
        [
            np.concatenate([W2.T, z64], axis=0),
            np.concatenate([z64, W2.T], axis=0),
            W3.T[:, 0:128],
            W3.T[:, 128:256],
        ],
        axis=1,
    ).astype(BF16)
    # v1t: V1.T is [256, 512] -> k-tiles stacked on cols [128, 2, 512]
    v1t_cols = V1.T.reshape(2, 128, 512).transpose(1, 0, 2).reshape(128, 1024)
    vblob = np.concatenate(
        [v1t_cols,
         V2.T.reshape(4, 128, 256).transpose(1, 0, 2).reshape(128, 1024),
         V3.T.reshape(2, 128, 40).transpose(1, 0, 2).reshape(128, 80)],
        axis=1,
    ).astype(BF16)
    cblob = np.zeros((128, 11), F32)
    cblob[:, 0] = np.concatenate([b1, b1])
    cblob[:, 1] = b2
    cblob[:, 2:4] = b3.reshape(2, 128).T
    cblob[:, 4:8] = c1.reshape(4, 128).T
    cblob[:, 8:10] = c2.reshape(2, 128).T
    cblob[0:40, 10] = c3

    shared = dict(wblob=wblob, w1blob=w1blob, vblob=vblob, cblob=cblob)

    Xv = X.reshape(B, D, M, DIN)
    in_maps = []
    for c in range(N_CORES):
        xts = np.empty((GROUPS_PER_CORE, DIN, M), F32)
        for gi in range(GROUPS_PER_CORE):
            g = 2 * c + gi
            bb, dd = g // D, g % D
            xts[gi] = Xv[bb, dd].T
        in_maps.append(dict(shared, xt=xts.astype(BF16)))
    return in_maps


def _run(inputs, trace=False):
    nc = _get_program()
    in_maps = _make_in_maps(inputs)
    res = run_bass_kernel_spmd(nc, in_maps, list(range(N_CORES)), trace=trace)
    ys = np.stack([res.results[c]["y"] for c in range(N_CORES)])  # [8, 2, 40]
    y16 = ys.reshape(B, D, 40)
    out = y16.max(axis=1).astype(F32)
    return out, res


def kernel(**inputs):
    out, _ = _run(inputs, trace=False)
    return out


# revision 14
# speedup vs baseline: 1.0200x; 1.0041x over previous
"""Trainium2 Bass kernel for BoostedPointPairNet2.

Model (per (b, d) group, m = 128 points, din = 3):
  H1(i,j) = relu(W1A @ x_j + W1B @ x_i + b1)          (64)
  H2(i,j) = relu(W2 @ H1 + b2)                        (128)
  G(i,j)  = W3 @ H2                                    (256, b3 deferred)
  P       = max_{i,j} G + b3                           (256)
  Y       = V3 @ relu(V2 @ relu(V1 @ P + c1) + c2) + c3  (40)
  out[b]  = max_d Y[b, d]

Sharding: 16 (b, d) groups over 8 cores, 2 groups per core. Weights
replicated. Each core returns its two groups' Y rows; the host does the
final max over d (the trivial "all-gather" of a (b, 40) output).

Per-core dataflow ("stacked pairs" layout): channels of two j-values are
stacked on the 128 SBUF partitions (j even -> partitions 0-63, j odd ->
64-127), so layer-1 (only 64 channels) runs at full partition width as ONE
fused DVE tensor_scalar op per j-pair: relu(v_stacked + (u_j + b1)).
L2 unstacks via block-padded weights [W2^T;0] / [0;W2^T]; L3 streams H2
through W3^T halves into PSUM.

The G max-reduction is the bottleneck: PSUM can only be read by the DVE
(0.96 GHz, the only engine with max) and ACT (1.2 GHz, no max). So G
tiles alternate between (a) direct DVE reduce_max from PSUM and (b) ACT
copy/cast to bf16 SBUF + DVE running tensor_tensor max at the 2x bf16
mode - balancing the two engines. Matmuls and big activations run in
bf16 (fp32 PSUM accumulation); the tiny F-MLP runs in fp32.
"""

import numpy as np
import ml_dtypes

import bass_rust
import concourse.bass as bass
import concourse.mybir as mybir
from concourse.tile import TileContext
from concourse.bass_utils import run_bass_kernel_spmd

BF16 = ml_dtypes.bfloat16
F32 = np.float32
DT = mybir.dt
ALU = mybir.AluOpType
AX = mybir.AxisListType
RELU = mybir.ActivationFunctionType.Relu

N_CORES = 8
B, N, DIN = 4, 512, 3
D = 4                    # boost factor
M = N // D               # 128 points per group
GROUPS_PER_CORE = 2
JP = M // 2              # 64 stacked j-pairs per group
NPAIR = JP // 4          # 16 pipeline iterations per group (4 jp each)
NGT = 2 * NPAIR          # 32 G-psum tiles per group
# G tiles with (t % DIRECT_MOD == 0) are reduced directly from PSUM by the
# DVE; the rest are ACT-copied to bf16 SBUF and running-maxed by the DVE.
DIRECT_MOD = 3
N_DIRECT = (NGT + DIRECT_MOD - 1) // DIRECT_MOD


# ---------------------------------------------------------------------------
# Workaround: this walrus build accepts at most ONE sync wait per instruction
# ("Too many sync wait commands"), while Tile emits up to 3. Hoist extra
# waits onto same-engine nop instructions inserted just before the offender
# (engines execute their queue in order, so the AND-wait semantics hold).
# ---------------------------------------------------------------------------
def _split_multi_waits(nc):
    seq = 0
    for fn in nc.m.functions:
        for bb in fn.blocks:
            new = []
            changed = False
            for ins in bb.instructions:
                si = ins.sync_info
                waits = list(si.on_wait) if si is not None and si.on_wait else []
                if len(waits) > 1:
                    changed = True
                    for w in waits[:-1]:
                        seq += 1
                        new.append(
                            mybir.InstNoOp(
                                name=f"I-wsplit-{seq}",
                                engine=ins.engine,
                                sync_info=bass_rust.SyncInfo(
                                    on_wait=[w], on_update=[]
                                ),
                            )
                        )
                    ins.sync_info = bass_rust.SyncInfo(
                        on_wait=[waits[-1]], on_update=list(si.on_update or [])
                    )
                new.append(ins)
            if changed:
                bb.instructions = new


# ---------------------------------------------------------------------------
# Device program
# ---------------------------------------------------------------------------
def _build_program():
    nc = bass.Bass(
        "TRN2", target_bir_lowering=False, debug=False, num_devices=N_CORES
    )

    xt = nc.declare_dram_parameter(
        "xt", [GROUPS_PER_CORE, DIN, M], DT.bfloat16, isOutput=False
    )
    # col 0:128 w2a=[W2T;0], 128:256 w2b=[0;W2T], 256:384 w3a, 384:512 w3b
    wblob = nc.declare_dram_parameter("wblob", [128, 512], DT.bfloat16, isOutput=False)
    # col 0:128 w1a_even=[W1AT|0], 128:256 w1a_odd=[0|W1AT], 256:384 w1b2=[W1BT|W1BT]
    w1blob = nc.declare_dram_parameter("w1blob", [DIN, 384], DT.bfloat16, isOutput=False)
    # v1t (2x512) | v2t (4x256) | v3t (2x40)
    vblob = nc.declare_dram_parameter("vblob", [128, 2128], DT.bfloat16, isOutput=False)
    # col 0 b1st, 1 b2c, 2:4 b3_2, 4:8 c1_4, 8:10 c2_2, 10 c3 (rows 0:40)
    cblob = nc.declare_dram_parameter("cblob", [128, 11], DT.float32, isOutput=False)
    y_out = nc.declare_dram_parameter(
        "y", [GROUPS_PER_CORE, 40], DT.float32, isOutput=True
    )

    with TileContext(nc) as tc:
        with (
            tc.tile_pool(name="singles", bufs=1) as singles,
            tc.tile_pool(name="xtp", bufs=2) as xtp,
            tc.tile_pool(name="v2p", bufs=2) as v2pool,
            tc.tile_pool(name="uup", bufs=2) as uupool,
            tc.tile_pool(name="h1p", bufs=3) as h1pool,
            tc.tile_pool(name="h2p", bufs=3) as h2pool,
            tc.tile_pool(name="gcp", bufs=4) as gcpool,
            tc.tile_pool(name="raccp", bufs=2) as raccpool,
            tc.tile_pool(name="fmlp", bufs=8) as fmlp,
            tc.tile_pool(name="psum", bufs=2, space="PSUM") as psum,
        ):
            # ---- load inputs; issue order = need order (2 HWDGE queues) ----
            sb_xts = []
            for g in range(GROUPS_PER_CORE):
                t = xtp.tile([DIN, M], DT.bfloat16)
                sb_xts.append(t)
            nc.sync.dma_start(out=sb_xts[0], in_=xt[0])
            sb_w1 = singles.tile([DIN, 384], DT.bfloat16, tag="w1blob")
            nc.scalar.dma_start(out=sb_w1, in_=w1blob[:, :])
            sb_c = singles.tile([128, 11], DT.float32, tag="cblob")
            nc.sync.dma_start(out=sb_c, in_=cblob[:, :])
            sb_w = singles.tile([128, 512], DT.bfloat16, tag="wblob")
            nc.scalar.dma_start(out=sb_w, in_=wblob[:, :])
            sb_v = singles.tile([128, 2128], DT.bfloat16, tag="vblob")
            nc.sync.dma_start(out=sb_v, in_=vblob[:, :])
            nc.scalar.dma_start(out=sb_xts[1], in_=xt[1])

            sb_w2a, sb_w2b = sb_w[:, 0:128], sb_w[:, 128:256]
            sb_w3a, sb_w3b = sb_w[:, 256:384], sb_w[:, 384:512]
            sb_w1a_e, sb_w1a_o = sb_w1[:, 0:128], sb_w1[:, 128:256]
            sb_w1b2 = sb_w1[:, 256:384]
            sb_b1st = sb_c[:, 0:1]
            sb_b2c = sb_c[:, 1:2]
            sb_b3_2 = sb_c[:, 2:4]
            sb_c1_4 = sb_c[:, 4:8]
            sb_c2_2 = sb_c[:, 8:10]
            sb_c3c = sb_c[0:40, 10:11]

            def v1t(k):  # [128, 512] f32, k in 0..1
                return sb_v[:, 512 * k : 512 * (k + 1)]

            def v2t(k):  # [128, 256] f32, k in 0..3
                return sb_v[:, 1024 + 256 * k : 1024 + 256 * (k + 1)]

            def v3t(k):  # [128, 40] f32, k in 0..1
                return sb_v[:, 2048 + 40 * k : 2048 + 40 * (k + 1)]

            for g in range(GROUPS_PER_CORE):
                # ---- per-group prep: stacked v (V2) and u+b1 (UU) ----
                sb_xt = sb_xts[g]
                xt_eo = sb_xt.rearrange("k (j two) -> k two j", two=2)

                v2ps = psum.tile([128, M], DT.float32, tag="l2")
                nc.tensor.matmul(v2ps, lhsT=sb_w1b2, rhs=sb_xt, start=True, stop=True)
                uups = psum.tile([128, JP], DT.float32, tag="g")
                nc.tensor.matmul(
                    uups, lhsT=sb_w1a_e, rhs=xt_eo[:, 0, :], start=True, stop=False
                )
                nc.tensor.matmul(
                    uups, lhsT=sb_w1a_o, rhs=xt_eo[:, 1, :], start=False, stop=True
                )
                sb_v2 = v2pool.tile([128, M], DT.bfloat16)
                nc.vector.tensor_copy(out=sb_v2, in_=v2ps)
                sb_uu = uupool.tile([128, JP], DT.float32)
                nc.vector.tensor_scalar_add(out=sb_uu, in0=uups, scalar1=sb_b1st)

                racc = raccpool.tile([128, 2, N_DIRECT], DT.float32)
                rbs = []
                rb_init = [False, False]
                for h in range(2):
                    rb = raccpool.tile([128, 1024], DT.bfloat16, tag=f"rb{h}")
                    rbs.append(rb)

                # ---- main pairwise pipeline: 4 j-pairs per iteration ----
                for it in range(NPAIR):
                    h1 = h1pool.tile([128, 512], DT.bfloat16)
                    for jj in range(4):
                        jp = it * 4 + jj
                        nc.vector.tensor_scalar(
                            out=h1[:, jj * M : (jj + 1) * M],
                            in0=sb_v2,
                            scalar1=sb_uu[:, jp : jp + 1],
                            scalar2=0.0,
                            op0=ALU.add,
                            op1=ALU.max,
                        )
                    # L2: weight-grouped matmuls into one 2-bank psum tile
                    l2ps = psum.tile([128, 1024], DT.float32, tag="l2")
                    nc.tensor.matmul(
                        l2ps[:, 0:256], lhsT=sb_w2a, rhs=h1[:, 0:256],
                        start=True, stop=True,
                    )
                    nc.tensor.matmul(
                        l2ps[:, 512:768], lhsT=sb_w2a, rhs=h1[:, 256:512],
                        start=True, stop=True,
                    )
                    nc.tensor.matmul(
                        l2ps[:, 256:512], lhsT=sb_w2b, rhs=h1[:, 0:256],
                        start=True, stop=True,
                    )
                    nc.tensor.matmul(
                        l2ps[:, 768:1024], lhsT=sb_w2b, rhs=h1[:, 256:512],
                        start=True, stop=True,
                    )
                    h2 = h2pool.tile([128, 1024], DT.bfloat16)
                    nc.scalar.activation(
                        out=h2, in_=l2ps, func=RELU, bias=sb_b2c, scale=1.0
                    )
                    # L3: weight-grouped into two G tiles (1024 pairs total)
                    gpa = psum.tile([128, 2, 512], DT.float32, tag="g")
                    gpb = psum.tile([128, 2, 512], DT.float32, tag="g")
                    nc.tensor.matmul(
                        gpa[:, 0, :], lhsT=sb_w3a, rhs=h2[:, 0:512],
                        start=True, stop=True,
                    )
                    nc.tensor.matmul(
                        gpb[:, 0, :], lhsT=sb_w3a, rhs=h2[:, 512:1024],
                        start=True, stop=True,
                    )
                    nc.tensor.matmul(
                        gpa[:, 1, :], lhsT=sb_w3b, rhs=h2[:, 0:512],
                        start=True, stop=True,
                    )
                    nc.tensor.matmul(
                        gpb[:, 1, :], lhsT=sb_w3b, rhs=h2[:, 512:1024],
                        start=True, stop=True,
                    )
                    for half, gp in enumerate((gpa, gpb)):
                        t = it * 2 + half
                        if t % DIRECT_MOD == 0:
                            nc.vector.reduce_max(
                                out=racc[:, :, t // DIRECT_MOD : t // DIRECT_MOD + 1],
                                in_=gp, axis=AX.X,
                            )
                        else:
                            gc = gcpool.tile([128, 1024], DT.bfloat16)
                            nc.scalar.copy(
                                out=gc, in_=gp.rearrange("p a b -> p (a b)")
                            )
                            rb = rbs[t % 2]
                            if not rb_init[t % 2]:
                                rb_init[t % 2] = True
                                nc.vector.tensor_copy(out=rb, in_=gc)
                            else:
                                nc.vector.tensor_tensor(
                                    out=rb, in0=gc, in1=rb, op=ALU.max
                                )

                # ---- P = max over accumulators, + b3; F MLP (fp32, N=1) ----
                pm1 = fmlp.tile([128, 2], DT.float32, tag="pm1")
                nc.vector.reduce_max(out=pm1, in_=racc, axis=AX.X)
                nc.vector.tensor_tensor(
                    out=rbs[0], in0=rbs[0], in1=rbs[1], op=ALU.max
                )
                pm2 = fmlp.tile([128, 2], DT.float32, tag="pm2")
                nc.vector.reduce_max(
                    out=pm2, in_=rbs[0].rearrange("p (a b) -> p a b", a=2), axis=AX.X
                )
                pmx = fmlp.tile([128, 2], DT.float32, tag="pmx")
                nc.vector.tensor_tensor(out=pmx, in0=pm1, in1=pm2, op=ALU.max)
                pb = fmlp.tile([128, 2], DT.bfloat16, tag="pb")
                nc.vector.tensor_tensor(out=pb, in0=pmx, in1=sb_b3_2, op=ALU.add)

                y1ps = psum.tile([128, 4], DT.float32, tag="l2")
                for mm in range(4):
                    for kk in range(2):
                        nc.tensor.matmul(
                            y1ps[:, mm : mm + 1],
                            lhsT=v1t(kk)[:, mm * 128 : (mm + 1) * 128],
                            rhs=pb[:, kk : kk + 1],
                            start=(kk == 0),
                            stop=(kk == 1),
                        )
                y1pre = fmlp.tile([128, 4], DT.float32, tag="y1pre")
                nc.vector.tensor_tensor(out=y1pre, in0=y1ps, in1=sb_c1_4, op=ALU.add)
                y1 = fmlp.tile([128, 4], DT.bfloat16, tag="y1")
                nc.vector.tensor_scalar_max(out=y1, in0=y1pre, scalar1=0.0)

                y2ps = psum.tile([128, 2], DT.float32, tag="l2")
                for mm in range(2):
                    for kk in range(4):
                        nc.tensor.matmul(
                            y2ps[:, mm : mm + 1],
                            lhsT=v2t(kk)[:, mm * 128 : (mm + 1) * 128],
                            rhs=y1[:, kk : kk + 1],
                            start=(kk == 0),
                            stop=(kk == 3),
                        )
                y2pre = fmlp.tile([128, 2], DT.float32, tag="y2pre")
                nc.vector.tensor_tensor(out=y2pre, in0=y2ps, in1=sb_c2_2, op=ALU.add)
                y2 = fmlp.tile([128, 2], DT.bfloat16, tag="y2")
                nc.vector.tensor_scalar_max(out=y2, in0=y2pre, scalar1=0.0)

                y3ps = psum.tile([40, 1], DT.float32, tag="g")
                for kk in range(2):
                    nc.tensor.matmul(
                        y3ps,
                        lhsT=v3t(kk)[:, 0:40],
                        rhs=y2[:, kk : kk + 1],
                        start=(kk == 0),
                        stop=(kk == 1),
                    )
                y3 = fmlp.tile([40, 1], DT.float32, tag="y3")
                nc.vector.tensor_scalar_add(out=y3, in0=y3ps, scalar1=sb_c3c)
                nc.sync.dma_start(out=y_out[g, :], in_=y3)

    _split_multi_waits(nc)
    return nc


# ---------------------------------------------------------------------------
# Host side
# ---------------------------------------------------------------------------
_NC_CACHE = None


def _get_program():
    global _NC_CACHE
    if _NC_CACHE is None:
        _NC_CACHE = _build_program()
    return _NC_CACHE


def _make_in_maps(inputs):
    X = np.asarray(inputs["X"], F32)
    W1 = np.asarray(inputs["W1"], F32)
    b1 = np.asarray(inputs["b1"], F32)
    W2 = np.asarray(inputs["W2"], F32)
    b2 = np.asarray(inputs["b2"], F32)
    W3 = np.asarray(inputs["W3"], F32)
    b3 = np.asarray(inputs["b3"], F32)
    V1 = np.asarray(inputs["V1"], F32)
    c1 = np.asarray(inputs["c1"], F32)
    V2 = np.asarray(inputs["V2"], F32)
    c2 = np.asarray(inputs["c2"], F32)
    V3 = np.asarray(inputs["V3"], F32)
    c3 = np.asarray(inputs["c3"], F32)

    W1A, W1B = W1[:, :DIN], W1[:, DIN:]
    z = np.zeros((DIN, 64), F32)
    w1blob = np.concatenate(
        [W1A.T, z, z, W1A.T, W1B.T, W1B.T], axis=1
    ).astype(BF16)
    z64 = np.zeros((64, 128), F32)
    wblob = np.concatenate(
        [
            np.concatenate([W2.T, z64], axis=0),
            np.concatenate([z64, W2.T], axis=0),
            W3.T[:, 0:128],
            W3.T[:, 128:256],
        ],
        axis=1,
    ).astype(BF16)
    # v1t: V1.T is [256, 512] -> k-tiles stacked on cols [128, 2, 512]
    v1t_cols = V1.T.reshape(2, 128, 512).transpose(1, 0, 2).reshape(128, 1024)
    vblob = np.concatenate(
        [v1t_cols,
         V2.T.reshape(4, 128, 256).transpose(1, 0, 2).reshape(128, 1024),
         V3.T.reshape(2, 128, 40).transpose(1, 0, 2).reshape(128, 80)],
        axis=1,
    ).astype(BF16)
    cblob = np.zeros((128, 11), F32)
    cblob[:, 0] = np.concatenate([b1, b1])
    cblob[:, 1] = b2
    cblob[:, 2:4] = b3.reshape(2, 128).T
    cblob[:, 4:8] = c1.reshape(4, 128).T
    cblob[:, 8:10] = c2.reshape(2, 128).T
    cblob[0:40, 10] = c3

    shared = dict(wblob=wblob, w1blob=w1blob, vblob=vblob, cblob=cblob)

    Xv = X.reshape(B, D, M, DIN)
    in_maps = []
    for c in range(N_CORES):
        xts = np.empty((GROUPS_PER_CORE, DIN, M), F32)
        for gi in range(GROUPS_PER_CORE):
            g = 2 * c + gi
            bb, dd = g // D, g % D
            xts[gi] = Xv[bb, dd].T
        in_maps.append(dict(shared, xt=xts.astype(BF16)))
    return in_maps


def _run(inputs, trace=False):
    nc = _get_program()
    in_maps = _make_in_maps(inputs)
    res = run_bass_kernel_spmd(nc, in_maps, list(range(N_CORES)), trace=trace)
    ys = np.stack([res.results[c]["y"] for c in range(N_CORES)])  # [8, 2, 40]
    y16 = ys.reshape(B, D, 40)
    out = y16.max(axis=1).astype(F32)
    return out, res


def kernel(**inputs):
    out, _ = _run(inputs, trace=False)
    return out


# revision 15
# speedup vs baseline: 1.0209x; 1.0009x over previous
"""Trainium2 Bass kernel for BoostedPointPairNet2.

Model (per (b, d) group, m = 128 points, din = 3):
  H1(i,j) = relu(W1A @ x_j + W1B @ x_i + b1)          (64)
  H2(i,j) = relu(W2 @ H1 + b2)                        (128)
  G(i,j)  = W3 @ H2                                    (256, b3 deferred)
  P       = max_{i,j} G + b3                           (256)
  Y       = V3 @ relu(V2 @ relu(V1 @ P + c1) + c2) + c3  (40)
  out[b]  = max_d Y[b, d]

Sharding: 16 (b, d) groups over 8 cores, 2 groups per core. Weights
replicated. Each core returns its two groups' Y rows; the host does the
final max over d (the trivial "all-gather" of a (b, 40) output).

Per-core dataflow ("stacked pairs" layout): channels of two j-values are
stacked on the 128 SBUF partitions (j even -> partitions 0-63, j odd ->
64-127), so layer-1 (only 64 channels) runs at full partition width as ONE
fused DVE tensor_scalar op per j-pair: relu(v_stacked + (u_j + b1)).
L2 unstacks via block-padded weights [W2^T;0] / [0;W2^T]; L3 streams H2
through W3^T halves into PSUM.

The G max-reduction is the bottleneck: PSUM can only be read by the DVE
(0.96 GHz, the only engine with max) and ACT (1.2 GHz, no max). So G
tiles alternate between (a) direct DVE reduce_max from PSUM and (b) ACT
copy/cast to bf16 SBUF + DVE running tensor_tensor max at the 2x bf16
mode - balancing the two engines. Matmuls and big activations run in
bf16 (fp32 PSUM accumulation); the tiny F-MLP runs in fp32.
"""

import numpy as np
import ml_dtypes

import bass_rust
import concourse.bass as bass
import concourse.mybir as mybir
from concourse.tile import TileContext
from concourse.bass_utils import run_bass_kernel_spmd

BF16 = ml_dtypes.bfloat16
F32 = np.float32
DT = mybir.dt
ALU = mybir.AluOpType
AX = mybir.AxisListType
RELU = mybir.ActivationFunctionType.Relu

N_CORES = 8
B, N, DIN = 4, 512, 3
D = 4                    # boost factor
M = N // D               # 128 points per group
GROUPS_PER_CORE = 2
JP = M // 2              # 64 stacked j-pairs per group
NPAIR = JP // 4          # 16 pipeline iterations per group (4 jp each)
NGT = 2 * NPAIR          # 32 G-psum tiles per group
# G tiles with (t % DIRECT_MOD == 0) are reduced directly from PSUM by the
# DVE; the rest are ACT-copied to bf16 SBUF and running-maxed by the DVE.
DIRECT_MOD = 3
N_DIRECT = (NGT + DIRECT_MOD - 1) // DIRECT_MOD


# ---------------------------------------------------------------------------
# Workaround: this walrus build accepts at most ONE sync wait per instruction
# ("Too many sync wait commands"), while Tile emits up to 3. Hoist extra
# waits onto same-engine nop instructions inserted just before the offender
# (engines execute their queue in order, so the AND-wait semantics hold).
# ---------------------------------------------------------------------------
def _split_multi_waits(nc):
    seq = 0
    for fn in nc.m.functions:
        for bb in fn.blocks:
            new = []
            changed = False
            for ins in bb.instructions:
                si = ins.sync_info
                waits = list(si.on_wait) if si is not None and si.on_wait else []
                if len(waits) > 1:
                    changed = True
                    for w in waits[:-1]:
                        seq += 1
                        new.append(
                            mybir.InstNoOp(
                                name=f"I-wsplit-{seq}",
                                engine=ins.engine,
                                sync_info=bass_rust.SyncInfo(
                                    on_wait=[w], on_update=[]
                                ),
                            )
                        )
                    ins.sync_info = bass_rust.SyncInfo(
                        on_wait=[waits[-1]], on_update=list(si.on_update or [])
                    )
                new.append(ins)
            if changed:
                bb.instructions = new


# ---------------------------------------------------------------------------
# Device program
# ---------------------------------------------------------------------------
def _build_program():
    nc = bass.Bass(
        "TRN2", target_bir_lowering=False, debug=False, num_devices=N_CORES
    )

    xt = nc.declare_dram_parameter(
        "xt", [GROUPS_PER_CORE, DIN, M], DT.bfloat16, isOutput=False
    )
    # col 0:128 w2a=[W2T;0], 128:256 w2b=[0;W2T], 256:384 w3a, 384:512 w3b
    wblob = nc.declare_dram_parameter("wblob", [128, 512], DT.bfloat16, isOutput=False)
    # col 0:128 w1a_even=[W1AT|0], 128:256 w1a_odd=[0|W1AT], 256:384 w1b2=[W1BT|W1BT]
    w1blob = nc.declare_dram_parameter("w1blob", [DIN, 384], DT.bfloat16, isOutput=False)
    # v1t (2x512) | v2t (4x256) | v3t (2x40)
    vblob = nc.declare_dram_parameter("vblob", [128, 2128], DT.bfloat16, isOutput=False)
    # col 0 b1st, 1 b2c, 2:4 b3_2, 4:8 c1_4, 8:10 c2_2, 10 c3 (rows 0:40)
    cblob = nc.declare_dram_parameter("cblob", [128, 11], DT.float32, isOutput=False)
    y_out = nc.declare_dram_parameter(
        "y", [GROUPS_PER_CORE, 40], DT.float32, isOutput=True
    )

    with TileContext(nc) as tc:
        with (
            tc.tile_pool(name="singles", bufs=1) as singles,
            tc.tile_pool(name="xtp", bufs=2) as xtp,
            tc.tile_pool(name="v2p", bufs=2) as v2pool,
            tc.tile_pool(name="uup", bufs=2) as uupool,
            tc.tile_pool(name="h1p", bufs=4) as h1pool,
            tc.tile_pool(name="h2p", bufs=4) as h2pool,
            tc.tile_pool(name="gcp", bufs=6) as gcpool,
            tc.tile_pool(name="raccp", bufs=2) as raccpool,
            tc.tile_pool(name="fmlp", bufs=8) as fmlp,
            tc.tile_pool(name="psum", bufs=2, space="PSUM") as psum,
        ):
            # ---- load inputs; issue order = need order (2 HWDGE queues) ----
            sb_xts = []
            for g in range(GROUPS_PER_CORE):
                t = xtp.tile([DIN, M], DT.bfloat16)
                sb_xts.append(t)
            nc.sync.dma_start(out=sb_xts[0], in_=xt[0])
            sb_w1 = singles.tile([DIN, 384], DT.bfloat16, tag="w1blob")
            nc.scalar.dma_start(out=sb_w1, in_=w1blob[:, :])
            sb_c = singles.tile([128, 11], DT.float32, tag="cblob")
            nc.sync.dma_start(out=sb_c, in_=cblob[:, :])
            sb_w = singles.tile([128, 512], DT.bfloat16, tag="wblob")
            nc.scalar.dma_start(out=sb_w, in_=wblob[:, :])
            sb_v = singles.tile([128, 2128], DT.bfloat16, tag="vblob")
            nc.sync.dma_start(out=sb_v, in_=vblob[:, :])
            nc.scalar.dma_start(out=sb_xts[1], in_=xt[1])

            sb_w2a, sb_w2b = sb_w[:, 0:128], sb_w[:, 128:256]
            sb_w3a, sb_w3b = sb_w[:, 256:384], sb_w[:, 384:512]
            sb_w1a_e, sb_w1a_o = sb_w1[:, 0:128], sb_w1[:, 128:256]
            sb_w1b2 = sb_w1[:, 256:384]
            sb_b1st = sb_c[:, 0:1]
            sb_b2c = sb_c[:, 1:2]
            sb_b3_2 = sb_c[:, 2:4]
            sb_c1_4 = sb_c[:, 4:8]
            sb_c2_2 = sb_c[:, 8:10]
            sb_c3c = sb_c[0:40, 10:11]

            def v1t(k):  # [128, 512] f32, k in 0..1
                return sb_v[:, 512 * k : 512 * (k + 1)]

            def v2t(k):  # [128, 256] f32, k in 0..3
                return sb_v[:, 1024 + 256 * k : 1024 + 256 * (k + 1)]

            def v3t(k):  # [128, 40] f32, k in 0..1
                return sb_v[:, 2048 + 40 * k : 2048 + 40 * (k + 1)]

            for g in range(GROUPS_PER_CORE):
                # ---- per-group prep: stacked v (V2) and u+b1 (UU) ----
                sb_xt = sb_xts[g]
                xt_eo = sb_xt.rearrange("k (j two) -> k two j", two=2)

                v2ps = psum.tile([128, M], DT.float32, tag="l2")
                nc.tensor.matmul(v2ps, lhsT=sb_w1b2, rhs=sb_xt, start=True, stop=True)
                uups = psum.tile([128, JP], DT.float32, tag="g")
                nc.tensor.matmul(
                    uups, lhsT=sb_w1a_e, rhs=xt_eo[:, 0, :], start=True, stop=False
                )
                nc.tensor.matmul(
                    uups, lhsT=sb_w1a_o, rhs=xt_eo[:, 1, :], start=False, stop=True
                )
                sb_v2 = v2pool.tile([128, M], DT.bfloat16)
                nc.vector.tensor_copy(out=sb_v2, in_=v2ps)
                sb_uu = uupool.tile([128, JP], DT.float32)
                nc.vector.tensor_scalar_add(out=sb_uu, in0=uups, scalar1=sb_b1st)

                racc = raccpool.tile([128, 2, N_DIRECT], DT.float32)
                rbs = []
                rb_init = [False, False]
                for h in range(2):
                    rb = raccpool.tile([128, 1024], DT.bfloat16, tag=f"rb{h}")
                    rbs.append(rb)

                # ---- main pairwise pipeline: 4 j-pairs per iteration ----
                for it in range(NPAIR):
                    h1 = h1pool.tile([128, 512], DT.bfloat16)
                    for jj in range(4):
                        jp = it * 4 + jj
                        nc.vector.tensor_scalar(
                            out=h1[:, jj * M : (jj + 1) * M],
                            in0=sb_v2,
                            scalar1=sb_uu[:, jp : jp + 1],
                            scalar2=0.0,
                            op0=ALU.add,
                            op1=ALU.max,
                        )
                    # L2: weight-grouped matmuls into one 2-bank psum tile
                    l2ps = psum.tile([128, 1024], DT.float32, tag="l2")
                    nc.tensor.matmul(
                        l2ps[:, 0:256], lhsT=sb_w2a, rhs=h1[:, 0:256],
                        start=True, stop=True,
                    )
                    nc.tensor.matmul(
                        l2ps[:, 512:768], lhsT=sb_w2a, rhs=h1[:, 256:512],
                        start=True, stop=True,
                    )
                    nc.tensor.matmul(
                        l2ps[:, 256:512], lhsT=sb_w2b, rhs=h1[:, 0:256],
                        start=True, stop=True,
                    )
                    nc.tensor.matmul(
                        l2ps[:, 768:1024], lhsT=sb_w2b, rhs=h1[:, 256:512],
                        start=True, stop=True,
                    )
                    h2 = h2pool.tile([128, 1024], DT.bfloat16)
                    nc.scalar.activation(
                        out=h2, in_=l2ps, func=RELU, bias=sb_b2c, scale=1.0
                    )
                    # L3: weight-grouped into two G tiles (1024 pairs total)
                    gpa = psum.tile([128, 2, 512], DT.float32, tag="g")
                    gpb = psum.tile([128, 2, 512], DT.float32, tag="g")
                    nc.tensor.matmul(
                        gpa[:, 0, :], lhsT=sb_w3a, rhs=h2[:, 0:512],
                        start=True, stop=True,
                    )
                    nc.tensor.matmul(
                        gpb[:, 0, :], lhsT=sb_w3a, rhs=h2[:, 512:1024],
                        start=True, stop=True,
                    )
                    nc.tensor.matmul(
                        gpa[:, 1, :], lhsT=sb_w3b, rhs=h2[:, 0:512],
                        start=True, stop=True,
                    )
                    nc.tensor.matmul(
                        gpb[:, 1, :], lhsT=sb_w3b, rhs=h2[:, 512:1024],
                        start=True, stop=True,
                    )
                    for half, gp in enumerate((gpa, gpb)):
                        t = it * 2 + half
                        if t % DIRECT_MOD == 0:
                            nc.vector.reduce_max(
                                out=racc[:, :, t // DIRECT_MOD : t // DIRECT_MOD + 1],
                                in_=gp, axis=AX.X,
                            )
                        else:
                            gc = gcpool.tile([128, 1024], DT.bfloat16)
                            nc.scalar.copy(
                                out=gc, in_=gp.rearrange("p a b -> p (a b)")
                            )
                            rb = rbs[t % 2]
                            if not rb_init[t % 2]:
                                rb_init[t % 2] = True
                                nc.vector.tensor_copy(out=rb, in_=gc)
                            else:
                                nc.vector.tensor_tensor(
                                    out=rb, in0=gc, in1=rb, op=ALU.max
                                )

                # ---- P = max over accumulators, + b3; F MLP (fp32, N=1) ----
                pm1 = fmlp.tile([128, 2], DT.float32, tag="pm1")
                nc.vector.reduce_max(out=pm1, in_=racc, axis=AX.X)
                nc.vector.tensor_tensor(
                    out=rbs[0], in0=rbs[0], in1=rbs[1], op=ALU.max
                )
                pm2 = fmlp.tile([128, 2], DT.float32, tag="pm2")
                nc.vector.reduce_max(
                    out=pm2, in_=rbs[0].rearrange("p (a b) -> p a b", a=2), axis=AX.X
                )
                pmx = fmlp.tile([128, 2], DT.float32, tag="pmx")
                nc.vector.tensor_tensor(out=pmx, in0=pm1, in1=pm2, op=ALU.max)
                pb = fmlp.tile([128, 2], DT.bfloat16, tag="pb")
                nc.vector.tensor_tensor(out=pb, in0=pmx, in1=sb_b3_2, op=ALU.add)

                y1ps = psum.tile([128, 4], DT.float32, tag="l2")
                for mm in range(4):
                    for kk in range(2):
                        nc.tensor.matmul(
                            y1ps[:, mm : mm + 1],
                            lhsT=v1t(kk)[:, mm * 128 : (mm + 1) * 128],
                            rhs=pb[:, kk : kk + 1],
                            start=(kk == 0),
                            stop=(kk == 1),
                        )
                y1pre = fmlp.tile([128, 4], DT.float32, tag="y1pre")
                nc.vector.tensor_tensor(out=y1pre, in0=y1ps, in1=sb_c1_4, op=ALU.add)
                y1 = fmlp.tile([128, 4], DT.bfloat16, tag="y1")
                nc.vector.tensor_scalar_max(out=y1, in0=y1pre, scalar1=0.0)

                y2ps = psum.tile([128, 2], DT.float32, tag="l2")
                for mm in range(2):
                    for kk in range(4):
                        nc.tensor.matmul(
                            y2ps[:, mm : mm + 1],
                            lhsT=v2t(kk)[:, mm * 128 : (mm + 1) * 128],
                            rhs=y1[:, kk : kk + 1],
                            start=(kk == 0),
                            stop=(kk == 3),
                        )
                y2pre = fmlp.tile([128, 2], DT.float32, tag="y2pre")
                nc.vector.tensor_tensor(out=y2pre, in0=y2ps, in1=sb_c2_2, op=ALU.add)
                y2 = fmlp.tile([128, 2], DT.bfloat16, tag="y2")
                nc.vector.tensor_scalar_max(out=y2, in0=y2pre, scalar1=0.0)

                y3ps = psum.tile([40, 1], DT.float32, tag="g")
                for kk in range(2):
                    nc.tensor.matmul(
                        y3ps,
                        lhsT=v3t(kk)[:, 0:40],
                        rhs=y2[:, kk : kk + 1],
                        start=(kk == 0),
                        stop=(kk == 1),
                    )
                y3 = fmlp.tile([40, 1], DT.float32, tag="y3")
                nc.vector.tensor_scalar_add(out=y3, in0=y3ps, scalar1=sb_c3c)
                nc.sync.dma_start(out=y_out[g, :], in_=y3)

    _split_multi_waits(nc)
    return nc


# ---------------------------------------------------------------------------
# Host side
# ---------------------------------------------------------------------------
_NC_CACHE = None


def _get_program():
    global _NC_CACHE
    if _NC_CACHE is None:
        _NC_CACHE = _build_program()
    return _NC_CACHE


def _make_in_maps(inputs):
    X = np.asarray(inputs["X"], F32)
    W1 = np.asarray(inputs["W1"], F32)
    b1 = np.asarray(inputs["b1"], F32)
    W2 = np.asarray(inputs["W2"], F32)
    b2 = np.asarray(inputs["b2"], F32)
    W3 = np.asarray(inputs["W3"], F32)
    b3 = np.asarray(inputs["b3"], F32)
    V1 = np.asarray(inputs["V1"], F32)
    c1 = np.asarray(inputs["c1"], F32)
    V2 = np.asarray(inputs["V2"], F32)
    c2 = np.asarray(inputs["c2"], F32)
    V3 = np.asarray(inputs["V3"], F32)
    c3 = np.asarray(inputs["c3"], F32)

    W1A, W1B = W1[:, :DIN], W1[:, DIN:]
    z = np.zeros((DIN, 64), F32)
    w1blob = np.concatenate(
        [W1A.T, z, z, W1A.T, W1B.T, W1B.T], axis=1
    ).astype(BF16)
    z64 = np.zeros((64, 128), F32)
    wblob = np.concatenate(
        [
            np.concatenate([W2.T, z64], axis=0),
            np.concatenate([z64, W2.T], axis=0),
            W3.T[:, 0:128],
            W3.T[:, 128:256],
        ],
        axis=1,
    ).astype(BF16)
    # v1t: V1.T is [256, 512] -> k-tiles stacked on cols [128, 2, 512]
    v1t_cols = V1.T.reshape(2, 128, 512).transpose(1, 0, 2).reshape(128, 1024)
    vblob = np.concatenate(
        [v1t_cols,
         V2.T.reshape(4, 128, 256).transpose(1, 0, 2).reshape(128, 1024),
         V3.T.reshape(2, 128, 40).transpose(1, 0, 2).reshape(128, 80)],
        axis=1,
    ).astype(BF16)
    cblob = np.zeros((128, 11), F32)
    cblob[:, 0] = np.concatenate([b1, b1])
    cblob[:, 1] = b2
    cblob[:, 2:4] = b3.reshape(2, 128).T
    cblob[:, 4:8] = c1.reshape(4, 128).T
    cblob[:, 8:10] = c2.reshape(2, 128).T
    cblob[0:40, 10] = c3

    shared = dict(wblob=wblob, w1blob=w1blob, vblob=vblob, cblob=cblob)

    Xv = X.reshape(B, D, M, DIN)
    in_maps = []
    for c in range(N_CORES):
        xts = np.empty((GROUPS_PER_CORE, DIN, M), F32)
        for gi in range(GROUPS_PER_CORE):
            g = 2 * c + gi
            bb, dd = g // D, g % D
            xts[gi] = Xv[bb, dd].T
        in_maps.append(dict(shared, xt=xts.astype(BF16)))
    return in_maps


def _run(inputs, trace=False):
    nc = _get_program()
    in_maps = _make_in_maps(inputs)
    res = run_bass_kernel_spmd(nc, in_maps, list(range(N_CORES)), trace=trace)
    ys = np.stack([res.results[c]["y"] for c in range(N_CORES)])  # [8, 2, 40]
    y16 = ys.reshape(B, D, 40)
    out = y16.max(axis=1).astype(F32)
    return out, res


def kernel(**inputs):
    out, _ = _run(inputs, trace=False)
    return out
